# revision 87
# baseline (speedup 1.0000x reference)
"""MetaPathAggregator kernel V5 — pair-packed transformed tables (Pool path)
+ raw HBM dma_gather (DMA path).

Math (linear collapse): out[t] = sum_k feat_k[i_k[t]] @ M_k with
  M0 = [0.5*Wdrug^T | 0.125*Wdd^T Wdis^T]   (mi)
  M1 = [0.25*Wdrug^T | 0.125*Wdd^T Wdis^T]  (g1)
  M2 = [0.125*Wdg^T Wdrug^T | 0.25*Wdis^T]  (g2)
  M3 = [0.125*Wdg^T Wdrug^T | 0.5*Wdis^T]   (dr)
M_k are folded on the host (weight-only preprocessing); features ship as bf16
in both row-major (feat_all, for the DMA path) and feature-major (feat_t, for
the on-chip table build) layouts.

Per core (TOK=16384 tokens = 1/8 of the n_pairs axis, data-parallel):
- Table build: T_k = feat_k @ M_k via 4 matmuls per 512-row batch
  (lhsT = M_k even/odd column split, rhs = feat_t slice), packed as bf16
  pairs (2c, 2c+1) in f32 containers: a_pack parts 0:64 = T0, 64:128 = T1;
  b_pack = T2|T3.  No PE transposes needed.
- A-path (A_TOK tokens, chunks per CHUNKS_A): one ap_gather per container
  per chunk (idx streams i0/i1 resp i2/i3 per partition half, 16-wrapped,
  x4 replicated).  Reduce: PSUM accumulate with stacked identity
  (psum[c] = S[c] + S[64+c]) over ga+gb, two 256-token blocks stacked in
  psum partition halves; ONE full [128,512] psum->bf16 copy per block pair
  (ACT/DVE alternating); one store per chunk.  The last chunk runs its ga
  matmuls during the gb gather (two-phase) to shorten the tail.
- D-path (D_TOK tokens, 2 halves): dma_gather raw bf16 rows straight from
  HBM feat_all [3072,128] (row offset selects the table), transpose mode ->
  g[f-part, slot*DH+tok].  Reduce: 4 matmuls lhsT=M_k per 512-token psum
  chunk; copy; store.  No staged table, no prep dependency; desc-gen for
  half 0 runs during the loads, half 1 early in the A loop.
- Scheduling: Pool gathers and main-loop PE matmul groups are dep-chained to
  emission order (keeps the in-order sequencers from head-of-line blocking on
  the D transfers and keeps PE dispatch just-in-time, i.e. warm p-state).
- Outputs bf16 [out_feat, token]; host transposes + upcasts (layout only).
"""

import numpy as np
import ml_dtypes

P = 128
F = 128
H = 128
HH = 64
R = 1024                 # padded table rows (indices < 1000)
NT = R // P              # 8 row-tiles per table
N_CORES = 8
B_PAIRS = 1024
BAG = 128
TOK = B_PAIRS * BAG // N_CORES   # 16384
CHUNKS_A = (1024, 1024, 2048, 2048, 2048, 2048, 1024, 1024)
CH_MAX = max(CHUNKS_A)
A_TOK = sum(CHUNKS_A)            # 12288
D_TOK = TOK - A_TOK              # 4096
DH = D_TOK // 2                  # tokens per dma_gather (2048)
CH_D = 512                       # D-path psum chunk
RED = 512                        # A-path psum reduce chunk (bf16 cols)
GA_BUFS = 3
STG_BUFS = 8
PKW = 512
APS_BUFS = 6
DPS_BUFS = 2
D2GEN_AFTER = 0                  # emit d2 desc-gen after this A-chunk
DH0_AFTER = 1                    # emit d_half(0) after this A-chunk
DH1_AFTER = 3                    # emit d_half(1) after this A-chunk

# idx tile columns (int16, [128, IDX_COLS])
IDX_A0 = 0                       # ga streams (i0 | i1)
IDX_B0 = A_TOK // 16             # gb streams (i2 | i3)
IDX_D1 = 2 * (A_TOK // 16)
DC = 4 * DH // 16                # 512 cols per dma_gather
IDX_D2 = IDX_D1 + DC
IDX_COLS = IDX_D2 + DC

_CACHE = {}


def _build_module():
    import contextlib

    import concourse.bacc as bacc
    import concourse.mybir as mybir
    import concourse.tile as tile
    from concourse.masks import make_identity

    f32 = mybir.dt.float32
    bf16 = mybir.dt.bfloat16
    i16 = mybir.dt.int16
    Copy = mybir.ActivationFunctionType.Copy

    nc = bacc.Bacc("TRN2", dynamic_dma_scratch_size=32768)

    feat_in = nc.dram_tensor("feat_all", [3 * R, F], bf16, kind="ExternalInput")
    featt_in = nc.dram_tensor("feat_t", [F, 3 * R], bf16, kind="ExternalInput")
    # host-folded collapsed matrices: m_eo[:, k, j, :] = M_k[:, j::2] (bf16),
    # mf[:, k, :] = M_k
    meo_in = nc.dram_tensor("m_eo", [F, 4, 2, HH], bf16, kind="ExternalInput")
    mf_in = nc.dram_tensor("m_full", [F, 4, H], bf16, kind="ExternalInput")
    idx_in = nc.dram_tensor("idx", [P, IDX_COLS], i16, kind="ExternalInput")
    out_p = nc.dram_tensor("out_p", [P, A_TOK], bf16, kind="ExternalOutput")
    out_d = nc.dram_tensor("out_d", [P, D_TOK], bf16, kind="ExternalOutput")

    with tile.TileContext(nc) as tc:
        with (
            tc.tile_pool(name="const", bufs=1) as cpool,
            tc.tile_pool(name="main", bufs=2) as mpool,
        ):
            prep_ps = contextlib.ExitStack()
            pkpool = prep_ps.enter_context(
                tc.tile_pool(name="pkps", bufs=2, space="PSUM"))

            # ---------------- loads ----------------
            # feat_t is feature-major (host-transposed): fat[f, t*1024 + r]
            idx = cpool.tile([P, IDX_COLS], i16, tag="idx")
            fat = cpool.tile([F, 3 * R], bf16, tag="fat")

            def load_feat(t):
                nc.sync.dma_start(fat[:, t * R:(t + 1) * R],
                                  featt_in[:, t * R:(t + 1) * R])

            meo = cpool.tile([F, 4, 2, HH], bf16, tag="meo")
            mf = cpool.tile([F, 4, H], bf16, tag="mf")
            nc.sync.dma_start(meo[:], meo_in[:, :, :, :])
            nc.sync.dma_start(idx[:, IDX_D1:IDX_D2], idx_in[:, IDX_D1:IDX_D2])
            load_feat(0)        # mi
            load_feat(1)        # ge
            load_feat(2)        # dr
            nc.sync.dma_start(idx[:, IDX_A0:IDX_D1], idx_in[:, IDX_A0:IDX_D1])
            nc.sync.dma_start(idx[:, IDX_D2:IDX_COLS], idx_in[:, IDX_D2:IDX_COLS])
            nc.sync.dma_start(mf[:], mf_in[:, :, :])
            m_ev = {k: meo[:, k, 0, :] for k in range(4)}
            m_od = {k: meo[:, k, 1, :] for k in range(4)}
            m_full = {k: mf[:, k, :] for k in range(4)}

            # ---------------- D-path gathers (raw rows straight from HBM) ---
            # d1 emitted now (desc-gen runs during prep); d2 emitted mid-loop.
            g_d = []
            for half in range(2):
                g = cpool.tile([P, 1, 4 * DH], bf16, tag=f"gd{half}")
                g_d.append(g)

            def d_gather(half):
                o = (IDX_D1, IDX_D2)[half]
                return nc.gpsimd.dma_gather(
                    g_d[half][:], feat_in[:, :], idx[:, o:o + DC],
                    4 * DH, 4 * DH, F,
                    transpose=True, single_packet=False,
                )

            d_gather(0)

            # ---------------- constants ----------------
            ident = cpool.tile([P, P], f32, tag="ident")
            make_identity(nc, ident[:])
            # stacked identity [128, 64] bf16: I2[k, m] = (k % 64 == m)
            i2 = cpool.tile([P, HH], bf16, tag="i2")
            nc.vector.tensor_copy(out=i2[0:HH, :], in_=ident[0:HH, 0:HH])
            nc.sync.dma_start(i2[HH:P, :], i2[0:HH, :])

            # containers: a_pack = T0|T1 pairs, b_pack = T2|T3
            W = PKW
            base = {"mi": 0, "ge": R, "dr": 2 * R}
            a_pack = cpool.tile([P, R], f32, tag="apack")
            b_pack = cpool.tile([P, R], f32, tag="bpack")
            packs = {"a": (a_pack, 0, 1, "mi", "ge"),
                     "b": (b_pack, 2, 3, "ge", "dr")}

            def build_pack(which, b):
                dest, klo, khi, nlo, nhi = packs[which]
                rs = slice(b * W, (b + 1) * W)
                flo = fat[:, base[nlo] + b * W:base[nlo] + (b + 1) * W]
                fhi = fat[:, base[nhi] + b * W:base[nhi] + (b + 1) * W]
                pe_ps = pkpool.tile([P, W], f32, tag="pkev", name=f"pe_{which}_{b}")
                po_ps = pkpool.tile([P, W], f32, tag="pkod", name=f"po_{which}_{b}")
                nc.tensor.matmul(out=pe_ps[0:HH, :], lhsT=m_ev[klo],
                                 rhs=flo, start=True, stop=True)
                nc.tensor.matmul(out=pe_ps[HH:P, :], lhsT=m_ev[khi],
                                 rhs=fhi, start=True, stop=True)
                nc.tensor.matmul(out=po_ps[0:HH, :], lhsT=m_od[klo],
                                 rhs=flo, start=True, stop=True)
                nc.tensor.matmul(out=po_ps[HH:P, :], lhsT=m_od[khi],
                                 rhs=fhi, start=True, stop=True)
                dv = dest[:].bitcast(bf16).rearrange("p (r two) -> p r two", two=2)
                nc.scalar.activation(out=dv[:, rs, 0], in_=pe_ps[:], func=Copy)
                nc.vector.tensor_copy(out=dv[:, rs, 1], in_=po_ps[:])

            for b in range(R // W):
                build_pack("a", b)
            for b in range(R // W):
                build_pack("b", b)

            prep_ps.close()
            main_ps = contextlib.ExitStack()
            apool = main_ps.enter_context(
                tc.tile_pool(name="aps", bufs=APS_BUFS, space="PSUM"))
            dpool = main_ps.enter_context(
                tc.tile_pool(name="dps", bufs=DPS_BUFS, space="PSUM"))

            # ---------------- main loops ----------------
            from concourse.tile_rust import add_dep_helper

            calt = [0]
            pool_chain = []     # pin Pool-engine order to emission order
            pe_chain = [None, None]   # [first-of-prev-group, last-of-prev-group]

            def chain(instr):
                if pool_chain:
                    add_dep_helper(instr.ins, pool_chain[-1].ins,
                                   reason="pool order")
                pool_chain.append(instr)

            def pe_group(mms):
                """Pin PE stream order: first mm of this psum group depends on
                the last mm of the previous group."""
                if pe_chain[1] is not None:
                    add_dep_helper(mms[0].ins, pe_chain[1].ins,
                                   reason="pe order")
                pe_chain[1] = mms[-1]

            def a_chunk(c, off, size):
                cols = slice(IDX_A0 + off // 16, IDX_A0 + (off + size) // 16)
                colsb = slice(IDX_B0 + off // 16, IDX_B0 + (off + size) // 16)
                ga_f = mpool.tile([P, CH_MAX], f32, tag="ga", name=f"ga{c}", bufs=GA_BUFS)
                ga = ga_f[:, :size]
                chain(nc.gpsimd.ap_gather(ga, a_pack[:], idx[:, cols], P, R, 1, size))
                gb_f = mpool.tile([P, CH_MAX], f32, tag="gb", name=f"gb{c}", bufs=GA_BUFS)
                gb = gb_f[:, :size]
                chain(nc.gpsimd.ap_gather(gb, b_pack[:], idx[:, colsb], P, R, 1, size))
                ga_bf = ga.bitcast(bf16)
                gb_bf = gb.bitcast(bf16)
                stg_f = mpool.tile([P, CH_MAX], bf16, tag="stg", name=f"stg{c}",
                                   bufs=STG_BUFS)
                stg = stg_f[:, :size]
                nred = 2 * size // RED
                last = c == len(CHUNKS_A) - 1
                if last:
                    # two-phase: all ga matmuls run while gb still gathers, so
                    # only gb matmuls + copies + store trail the final gather
                    pss = []
                    for j in range(0, nred, 2):
                        ps = apool.tile([P, RED], f32, tag="aps",
                                        name=f"aps{c}_{j}")
                        pss.append(ps)
                        mms = []
                        for h, jx in ((0, j), (1, j + 1)):
                            hs = slice(h * HH, (h + 1) * HH)
                            sl = slice(jx * RED, (jx + 1) * RED)
                            mms.append(nc.tensor.matmul(
                                out=ps[hs, :], lhsT=i2[:],
                                rhs=ga_bf[:, sl], start=True, stop=False))
                        pe_group(mms)
                for j in range(0, nred, 2):
                    jj = j // 2
                    cs = slice(jj * RED, (jj + 1) * RED)
                    mms = []
                    if last:
                        ps = pss[jj]
                    else:
                        ps = apool.tile([P, RED], f32, tag="aps",
                                        name=f"aps{c}_{j}")
                        for h, jx in ((0, j), (1, j + 1)):
                            hs = slice(h * HH, (h + 1) * HH)
                            sl = slice(jx * RED, (jx + 1) * RED)
                            mms.append(nc.tensor.matmul(
                                out=ps[hs, :], lhsT=i2[:],
                                rhs=ga_bf[:, sl], start=True, stop=False))
                    for h, jx in ((0, j), (1, j + 1)):
                        hs = slice(h * HH, (h + 1) * HH)
                        sl = slice(jx * RED, (jx + 1) * RED)
                        mms.append(nc.tensor.matmul(
                            out=ps[hs, :], lhsT=i2[:],
                            rhs=gb_bf[:, sl], start=False, stop=True))
                    pe_group(mms)
                    if calt[0] % 2 == 0:
                        nc.scalar.activation(out=stg[:, cs], in_=ps[:], func=Copy)
                    else:
                        nc.vector.tensor_copy(out=stg[:, cs], in_=ps[:])
                    calt[0] += 1
                if c == len(CHUNKS_A) - 1:
                    # split the final store so the first half drains while the
                    # second half is still being reduced
                    h = size // 2
                    nc.sync.dma_start(out_p[:, off:off + h], stg[:, :h])
                    nc.sync.dma_start(out_p[:, off + h:off + size], stg[:, h:])
                else:
                    nc.sync.dma_start(out_p[:, off:off + size], stg)

            def d_half(half):
                g = g_d[half]
                stg = mpool.tile([P, DH], bf16, tag="stgd", name=f"stgd{half}")
                for j in range(DH // CH_D):
                    js = slice(j * CH_D, (j + 1) * CH_D)
                    ps = dpool.tile([P, CH_D], f32, tag="dps", name=f"dps{half}_{j}")
                    mms = []
                    for k in range(4):
                        sl = slice(k * DH + j * CH_D, k * DH + (j + 1) * CH_D)
                        mms.append(nc.tensor.matmul(
                            out=ps[:], lhsT=m_full[k],
                            rhs=g[:, 0, sl], start=(k == 0), stop=(k == 3)))
                    pe_group(mms)
                    if calt[0] % 2 == 0:
                        nc.scalar.activation(out=stg[:, js], in_=ps[:], func=Copy)
                    else:
                        nc.vector.tensor_copy(out=stg[:, js], in_=ps[:])
                    calt[0] += 1
                nc.sync.dma_start(out_d[:, half * DH:(half + 1) * DH], stg[:])

            offs = []
            o = 0
            for s in CHUNKS_A:
                offs.append(o)
                o += s
            n = len(CHUNKS_A)
            for c in range(n):
                a_chunk(c, offs[c], CHUNKS_A[c])
                if c == D2GEN_AFTER:
                    gi2 = d_gather(1)
                    add_dep_helper(gi2.ins, pool_chain[-1].ins,
                                   reason="pool order")
                if c == DH0_AFTER:
                    d_half(0)
                if c == DH1_AFTER:
                    d_half(1)

            main_ps.close()

    nc.compile()
    return nc


def _wrap16(v):
    """token j -> [j % 16, j // 16] layout."""
    return np.ascontiguousarray(v.reshape(-1, 16).T)


def _prep_inputs(feat_miRNA, feat_gene, feat_drug, W_drug_disease, W_disease_drug,
                 W_drug, W_dis, mp_ins):
    def pad_rows(a):
        a = np.asarray(a, dtype=np.float32)
        out = np.zeros((R, a.shape[1]), dtype=np.float32)
        out[: min(R, a.shape[0])] = a[:R]
        return out

    feat_all = np.concatenate(
        [pad_rows(feat_miRNA), pad_rows(feat_gene), pad_rows(feat_drug)]
    ).astype(ml_dtypes.bfloat16)
    feat_t = np.ascontiguousarray(feat_all.T)                     # [F, 3R]
    # fold the BiTrans weights into per-slot collapsed matrices M_k [F, H]
    wdd = np.asarray(W_drug_disease, np.float32)
    wdg = np.asarray(W_disease_drug, np.float32)
    wdrug = np.asarray(W_drug, np.float32)
    wdis = np.asarray(W_dis, np.float32)
    C = wdrug.T                       # [128, 64]
    D = wdis.T
    A = (wdis @ wdd).T                # Wdd^T @ Wdis^T
    B = (wdrug @ wdg).T               # Wdg^T @ Wdrug^T
    M = [np.concatenate([0.5 * C, 0.125 * A], axis=1),
         np.concatenate([0.25 * C, 0.125 * A], axis=1),
         np.concatenate([0.125 * B, 0.25 * D], axis=1),
         np.concatenate([0.125 * B, 0.5 * D], axis=1)]
    m_full = np.stack(M, axis=1).astype(ml_dtypes.bfloat16)       # [F, 4, H]
    m_eo = np.stack(
        [np.stack([Mk[:, 0::2], Mk[:, 1::2]], axis=1) for Mk in M],
        axis=1).astype(ml_dtypes.bfloat16)                        # [F, 4, 2, HH]
    m_full = np.ascontiguousarray(m_full)
    m_eo = np.ascontiguousarray(m_eo)

    mp = np.asarray(mp_ins)
    assert mp.shape == (B_PAIRS, BAG, 4), mp.shape

    in_maps = []
    for core in range(N_CORES):
        mp_core = mp[core * (B_PAIRS // N_CORES):(core + 1) * (B_PAIRS // N_CORES)]
        mp_core = mp_core.reshape(TOK, 4).astype(np.int16)
        i0, i1, i2, i3 = (mp_core[:, k] for k in range(4))
        idx = np.empty((P, IDX_COLS), dtype=np.int16)
        idx[0:HH, IDX_A0:IDX_B0] = np.tile(_wrap16(i0[:A_TOK]), (4, 1))
        idx[HH:P, IDX_A0:IDX_B0] = np.tile(_wrap16(i1[:A_TOK]), (4, 1))
        idx[0:HH, IDX_B0:IDX_D1] = np.tile(_wrap16(i2[:A_TOK]), (4, 1))
        idx[HH:P, IDX_B0:IDX_D1] = np.tile(_wrap16(i3[:A_TOK]), (4, 1))
        for half in range(2):
            t = slice(A_TOK + half * DH, A_TOK + (half + 1) * DH)
            d = np.concatenate(
                [i0[t], R + i1[t], R + i2[t], 2 * R + i3[t]]).astype(np.int16)
            o = (IDX_D1, IDX_D2)[half]
            idx[:, o:o + DC] = np.tile(_wrap16(d), (8, 1))
        in_maps.append({"feat_all": feat_all, "feat_t": feat_t, "m_eo": m_eo,
                        "m_full": m_full, "idx": idx})
    return in_maps


def _assemble(results):
    outs = []
    for r in results:
        op = np.asarray(r["out_p"]).astype(np.float32)      # [128, A_TOK]
        od = np.asarray(r["out_d"]).astype(np.float32)      # [128, D_TOK]
        # op[h*64+c, off + jj*512 + 2*s + l] = feat(2c+l) of token
        #   off + jj*512 + h*256 + s
        parts = []
        off = 0
        for L in CHUNKS_A:
            a = op[:, off:off + L].reshape(2, HH, L // RED, RED // 2, 2)
            a = a.transpose(2, 0, 3, 1, 4).reshape(L, H)
            parts.append(a)
            off += L
        outs.append(np.concatenate(parts + [od.T], axis=0))
    return np.concatenate(outs, axis=0).reshape(B_PAIRS, BAG, H)


def _numpy_fallback(feat_miRNA, feat_gene, feat_drug, W_drug_disease,
                    W_disease_drug, W_drug, W_dis, mp_ins):
    mi = np.asarray(feat_miRNA, np.float32)[mp_ins[:, :, 0]]
    g1 = np.asarray(feat_gene, np.float32)[mp_ins[:, :, 1]]
    g2 = np.asarray(feat_gene, np.float32)[mp_ins[:, :, 2]]
    dr = np.asarray(feat_drug, np.float32)[mp_ins[:, :, 3]]
    wdd = np.asarray(W_drug_disease, np.float32)
    wdg = np.asarray(W_disease_drug, np.float32)
    wdrug = np.asarray(W_drug, np.float32)
    wdis = np.asarray(W_dis, np.float32)
    dis = ((((mi + g1) * 0.5) @ wdd.T + g2) * 0.5 + dr) * 0.5
    drug = ((((dr + g2) * 0.5) @ wdg.T + g1) * 0.5 + mi) * 0.5
    return np.concatenate([drug @ wdrug.T, dis @ wdis.T], axis=2)


def kernel(**inputs):
    mp = np.asarray(inputs["mp_ins"])
    if mp.max() >= R or mp.min() < 0:
        return _numpy_fallback(**inputs)

    from concourse.bass_utils import run_bass_kernel_spmd

    if "nc" not in _CACHE:
        _CACHE["nc"] = _build_module()
    nc = _CACHE["nc"]

    in_maps = _prep_inputs(**inputs)
    res = run_bass_kernel_spmd(nc, in_maps, core_ids=list(range(N_CORES)))
    return _assemble(res.results)


if __name__ == "__main__":
    import reference

    inputs = {k: np.asarray(v) for k, v in reference.setup_inputs().items()}
    expected = np.asarray(reference.reference(**inputs))
    actual = kernel(**inputs)
    rel = np.linalg.norm(actual - expected) / np.linalg.norm(expected)
    print("Relative error:", rel)

    from concourse.timeline_sim import TimelineSim
    print("TimelineSim:", TimelineSim(_CACHE["nc"], trace=False).simulate(), "ns")
